# revision 3
# baseline (speedup 1.0000x reference)
"""Trainium2 Bass kernel for the masked single-head AttentionBlock.

Contract: kernel(**inputs) takes the FULL unsharded inputs from
reference.setup_inputs() and returns (out, rel, raw) exactly like
reference.reference().

Sharding: 8 NeuronCores = 4 batches x 2 key-column halves. Core c handles
(b = c//2, half = c%2) and computes:
  qT[kk, l]   = (x[b] @ Wq + bq).T           (full l; duplicated per pair)
  kT[kk, m]   = (x[b, mhalf] @ Wk + bk).T    (its 1024 m columns)
  v[m, vc]    =  x[b, mhalf] @ Wv + bv
  S_T[m, l]   = k @ q.T      (scores transposed: softmax axis l = free axis)
  rel_T[m, l] = softmax_l( where(ts[l] < ts[m], -inf, S_T) / 32 )
  attnT[vc,l] = v.T @ rel_T                  (partial sum over its m-half)
Host: concatenates raw/rel halves over m (transposing back to [l, m]),
sums the two attn halves, and emits out = concat([x, attn], -1).

All matmuls run in bf16 with fp32 PSUM accumulation; raw/rel/attn leave
the chip as bf16 and are upcast to fp32 on the host. exp needs no
max-subtraction: scores/32 is ~N(0,1) and provably far from fp32 range.
"""
import sys

sys.path.insert(0, "/opt/trn_rl_repo")

import numpy as np
import ml_dtypes
import bass_rust
import concourse.bass as bass
import concourse.mybir as mybir
import concourse.tile as tile
from concourse.bass import ts as bts
from concourse.bass_utils import run_bass_kernel_spmd

BF16 = mybir.dt.bfloat16
F32 = mybir.dt.float32
AF = mybir.ActivationFunctionType
OP = mybir.AluOpType
NPBF16 = ml_dtypes.bfloat16

B, L, D, K, V = 4, 2048, 1024, 1024, 1024
P = 128
MH = 1024                                   # m columns per core
DC, KC, MC, VC = D // P, K // P, MH // P, V // P   # all 8
LB = L // 512                               # 4 rhs blocks over l
SCALE = 1.0 / 32.0                          # 1/sqrt(K)

# ---------------------------------------------------------------- walrus fix
# This container's walrus build rejects instructions carrying more than one
# sync-wait command. Hoist excess waits onto injected same-engine nops
# (program order on the engine preserves wait-then-execute semantics).
_WAITFIX_UID = [0]


def _fix_sync_waits(nc, limit=1):
    for fn in nc.m.functions:
        for blk in fn.blocks:
            out = []
            for inst in blk.instructions:
                si = inst.sync_info
                waits = list(si.on_wait) if si is not None else []
                if len(waits) > limit:
                    extra, keep = waits[:-limit], waits[-limit:]
                    for j in range(0, len(extra), limit):
                        _WAITFIX_UID[0] += 1
                        nop = bass_rust.InstNoOp(
                            name=f"waitfix-{_WAITFIX_UID[0]}",
                            engine=inst.engine, ins=[], outs=[],
                        )
                        nop.sync_info = bass_rust.SyncInfo(
                            on_wait=extra[j:j + limit], on_update=[])
                        out.append(nop)
                    inst.sync_info = bass_rust.SyncInfo(
                        on_wait=keep, on_update=list(si.on_update))
                out.append(inst)
            blk.instructions = out


# ---------------------------------------------------------------- bass build
def _build_nc():
    nc = bass.Bass()
    dp = nc.declare_dram_parameter
    xT = dp("xT", [D, L], BF16, isOutput=False)     # x[b].T, full l
    xTh = dp("xTh", [D, MH], BF16, isOutput=False)  # x[b].T, this half's m
    wq = dp("wq", [D, K], BF16, isOutput=False)
    wk = dp("wk", [D, K], BF16, isOutput=False)
    wv = dp("wv", [D, V], BF16, isOutput=False)
    bq = dp("bq_cols", [P, KC], F32, isOutput=False)
    bk = dp("bk_cols", [P, KC], F32, isOutput=False)
    bv = dp("bv_bcast", [P, V], F32, isOutput=False)
    tsb = dp("ts_bcast", [P, L], F32, isOutput=False)
    tsc = dp("ts_cols", [P, MC], F32, isOutput=False)
    rawT = dp("rawT", [MH, L], BF16, isOutput=True)
    relT = dp("relT", [MH, L], BF16, isOutput=True)
    attnT = dp("attnT", [V, L], BF16, isOutput=True)

    with tile.TileContext(nc) as tc:
        with (
            tc.tile_pool(name="const", bufs=1) as p_const,
            tc.tile_pool(name="qkv", bufs=1) as p_qkv,
            tc.tile_pool(name="relp", bufs=1) as p_rel,
            tc.tile_pool(name="psum", bufs=8, space="PSUM") as p_ps,
        ):
            tsb_sb = p_const.tile([P, L], F32)
            tsc_sb = p_const.tile([P, MC], F32)
            bq_sb = p_const.tile([P, KC], F32)
            bk_sb = p_const.tile([P, KC], F32)
            bv_sb = p_const.tile([P, V], F32)
            nc.sync.dma_start(tsb_sb[:], tsb[:])
            nc.sync.dma_start(tsc_sb[:], tsc[:])
            nc.sync.dma_start(bq_sb[:], bq[:])
            nc.sync.dma_start(bk_sb[:], bk[:])
            nc.sync.dma_start(bv_sb[:], bv[:])

            qT_sb = p_qkv.tile([P, KC, L], BF16)     # kk-chunks x l
            kT_sb = p_qkv.tile([P, KC, MH], BF16)    # kk-chunks x m
            v_sb = p_qkv.tile([P, MC, V], BF16)      # m-chunks x vc
            rel_sb = p_rel.tile([P, MC, L], BF16)    # m-chunks x l

            with tc.tile_pool(name="inp", bufs=1) as p_in:
                xT_sb = p_in.tile([P, DC, L], BF16)
                xTh_sb = p_in.tile([P, DC, MH], BF16)
                wq_sb = p_in.tile([P, DC, K], BF16)
                wk_sb = p_in.tile([P, DC, K], BF16)
                wv_sb = p_in.tile([P, DC, V], BF16)
                xT_ap = xT.ap().rearrange("(o p) l -> p o l", p=P)
                xTh_ap = xTh.ap().rearrange("(o p) m -> p o m", p=P)
                wq_ap = wq.ap().rearrange("(o p) k -> p o k", p=P)
                wk_ap = wk.ap().rearrange("(o p) k -> p o k", p=P)
                wv_ap = wv.ap().rearrange("(o p) k -> p o k", p=P)
                for dc in range(DC):  # chunked so first matmuls start early
                    nc.sync.dma_start(xT_sb[:, dc, :], xT_ap[:, dc, :])
                    nc.sync.dma_start(xTh_sb[:, dc, :], xTh_ap[:, dc, :])
                    nc.sync.dma_start(wq_sb[:, dc, :], wq_ap[:, dc, :])
                    nc.sync.dma_start(wk_sb[:, dc, :], wk_ap[:, dc, :])
                    nc.sync.dma_start(wv_sb[:, dc, :], wv_ap[:, dc, :])

                # Q projection: qT[kk, l] = wq[d, kk].T @ xT[d, l]  (+bq)
                for t in range(KC):
                    pss = [p_ps.tile([P, 512], F32, tag="ps", name="ps") for _ in range(LB)]
                    for dc in range(DC):
                        for lb in range(LB):
                            nc.tensor.matmul(
                                pss[lb][:], wq_sb[:, dc, bts(t, P)],
                                xT_sb[:, dc, bts(lb, 512)],
                                start=(dc == 0), stop=(dc == DC - 1))
                    for lb in range(LB):
                        nc.vector.tensor_scalar(
                            qT_sb[:, t, bts(lb, 512)], pss[lb][:],
                            bq_sb[:, t:t + 1], None, OP.add)

                # K projection: kT[kk, m] = wk[d, kk].T @ xTh[d, m]  (+bk)
                for t in range(KC):
                    pss = [p_ps.tile([P, 512], F32, tag="ps", name="ps") for _ in range(2)]
                    for dc in range(DC):
                        for mb in range(2):
                            nc.tensor.matmul(
                                pss[mb][:], wk_sb[:, dc, bts(t, P)],
                                xTh_sb[:, dc, bts(mb, 512)],
                                start=(dc == 0), stop=(dc == DC - 1))
                    for mb in range(2):
                        nc.vector.tensor_scalar(
                            kT_sb[:, t, bts(mb, 512)], pss[mb][:],
                            bk_sb[:, t:t + 1], None, OP.add)

                # V projection: v[m, vc] = xTh[d, m].T @ wv[d, vc]  (+bv)
                for mc in range(MC):
                    pss = [p_ps.tile([P, 512], F32, tag="ps", name="ps") for _ in range(2)]
                    for dc in range(DC):
                        for vb in range(2):
                            nc.tensor.matmul(
                                pss[vb][:], xTh_sb[:, dc, bts(mc, P)],
                                wv_sb[:, dc, bts(vb, 512)],
                                start=(dc == 0), stop=(dc == DC - 1))
                    for vb in range(2):
                        nc.vector.tensor_tensor(
                            v_sb[:, mc, bts(vb, 512)], pss[vb][:],
                            bv_sb[:, bts(vb, 512)], OP.add)

            with (
                tc.tile_pool(name="work", bufs=2) as p_work,
                tc.tile_pool(name="stat", bufs=4) as p_stat,
            ):
                # scores + masked softmax over l, one 128-row m-chunk at a time
                for mc in range(MC):
                    pss = [p_ps.tile([P, 512], F32, tag="ps", name="ps") for _ in range(LB)]
                    for t in range(KC):
                        for lb in range(LB):
                            nc.tensor.matmul(
                                pss[lb][:], kT_sb[:, t, bts(mc, P)],
                                qT_sb[:, t, bts(lb, 512)],
                                start=(t == 0), stop=(t == KC - 1))
                    raw_bf = p_work.tile([P, L], BF16, tag="rawbf")
                    e_sb = p_work.tile([P, L], F32, tag="e")
                    for lb in range(LB):
                        nc.vector.tensor_copy(raw_bf[:, bts(lb, 512)], pss[lb][:])
                        nc.scalar.activation(
                            e_sb[:, bts(lb, 512)], pss[lb][:], AF.Exp, scale=SCALE)
                    nc.sync.dma_start(rawT[bts(mc, P), :], raw_bf[:])
                    ev_sb = p_work.tile([P, L], F32, tag="ev")
                    rowsum = p_stat.tile([P, 1], F32, tag="rs")
                    nc.vector.scalar_tensor_tensor(
                        ev_sb[:], tsb_sb[:], tsc_sb[:, mc:mc + 1], e_sb[:],
                        OP.is_ge, OP.mult, accum_out=rowsum[:])
                    rinv = p_stat.tile([P, 1], F32, tag="ri")
                    nc.vector.reciprocal(rinv[:], rowsum[:])
                    nc.vector.tensor_scalar(
                        rel_sb[:, mc, :], ev_sb[:], rinv[:], None, OP.mult)
                    nc.sync.dma_start(relT[bts(mc, P), :], rel_sb[:, mc, :])

                # attention: attnT[vc, l] = sum_mc v[mc, vc].T @ rel_T[mc, l]
                for vc in range(VC):
                    pss = [p_ps.tile([P, 512], F32, tag="ps", name="ps") for _ in range(LB)]
                    for mc in range(MC):
                        for lb in range(LB):
                            nc.tensor.matmul(
                                pss[lb][:], v_sb[:, mc, bts(vc, P)],
                                rel_sb[:, mc, bts(lb, 512)],
                                start=(mc == 0), stop=(mc == MC - 1))
                    at_bf = p_work.tile([P, L], BF16, tag="atbf")
                    for lb in range(LB):
                        nc.scalar.copy(at_bf[:, bts(lb, 512)], pss[lb][:])
                    nc.sync.dma_start(attnT[bts(vc, P), :], at_bf[:])

    _fix_sync_waits(nc)
    return nc


_CACHE = {}


def _get_nc():
    if "nc" not in _CACHE:
        _CACHE["nc"] = _build_nc()
    return _CACHE["nc"]


def make_in_maps(x, time_steps, Wq, bq, Wk, bk, Wv, bv):
    x = np.asarray(x, dtype=np.float32)
    tsf = np.asarray(time_steps).astype(np.float32)
    xT = np.ascontiguousarray(np.asarray(x).transpose(0, 2, 1)).astype(NPBF16)
    wq_b = np.asarray(Wq, np.float32).astype(NPBF16)
    wk_b = np.asarray(Wk, np.float32).astype(NPBF16)
    wv_b = np.asarray(Wv, np.float32).astype(NPBF16)
    bq_cols = np.ascontiguousarray(
        np.asarray(bq, np.float32).reshape(KC, P).T)
    bk_cols = np.ascontiguousarray(
        np.asarray(bk, np.float32).reshape(KC, P).T)
    bv_bcast = np.ascontiguousarray(
        np.broadcast_to(np.asarray(bv, np.float32), (P, V)))
    in_maps = []
    for c in range(8):
        b, h = c // 2, c % 2
        in_maps.append({
            "xT": xT[b],
            "xTh": np.ascontiguousarray(xT[b][:, h * MH:(h + 1) * MH]),
            "wq": wq_b, "wk": wk_b, "wv": wv_b,
            "bq_cols": bq_cols, "bk_cols": bk_cols, "bv_bcast": bv_bcast,
            "ts_bcast": np.ascontiguousarray(
                np.broadcast_to(tsf[b], (P, L))),
            "ts_cols": np.ascontiguousarray(
                tsf[b, h * MH:(h + 1) * MH].reshape(MC, P).T),
        })
    return in_maps


def assemble(x, results):
    x = np.asarray(x, dtype=np.float32)
    raw = np.empty((B, L, L), np.float32)
    rel = np.empty((B, L, L), np.float32)
    attn = np.zeros((B, L, V), np.float32)
    for c in range(8):
        b, h = c // 2, c % 2
        r = results[c]
        raw[b][:, h * MH:(h + 1) * MH] = r["rawT"].astype(np.float32).T
        rel[b][:, h * MH:(h + 1) * MH] = r["relT"].astype(np.float32).T
        attn[b] += r["attnT"].astype(np.float32).T
    out = np.concatenate([x, attn], axis=2)
    return out, rel, raw


def kernel(x, time_steps, Wq, bq, Wk, bk, Wv, bv):
    in_maps = make_in_maps(x, time_steps, Wq, bq, Wk, bk, Wv, bv)
    res = run_bass_kernel_spmd(_get_nc(), in_maps, list(range(8)))
    return assemble(x, res.results)


# revision 6
# speedup vs baseline: 1.0224x; 1.0224x over previous
"""Trainium2 Bass kernel for the masked single-head AttentionBlock.

Contract: kernel(**inputs) takes the FULL unsharded inputs from
reference.setup_inputs() and returns (out, rel, raw) exactly like
reference.reference().

Sharding: 8 NeuronCores = 4 batches x 2 key-column halves. Core c handles
(b = c//2, half = c%2).

Math: q and k only ever appear through raw = q @ k.T, so with
M = Wk @ Wq.T (computed once on the host in fp32):
  raw.T[m, l] = x[m] @ M @ x[l].T  + col[m] + row[l]
where col[m] = x[m]@(Wk bq) + bk@bq and row[l] = x[l]@(Wq bk) are host-
computed rank-1 bias corrections (identically zero for this problem's
zero biases, but kept for generality: row is added to the score PSUM by
the DVE, col rides the existing per-partition bias slots of the raw-cast
and exp instructions).

Per core:
  gT[d', m]   = M.T @ xTh                    (128 matmuls)
  v[m, vc]    = xTh.T @ Wv + bv              (128)
  S_T[m, l]   = gT.T @ xT  (+row +col)       (256; softmax axis l is free)
  rel_T[m, l] = softmax_l(where(ts[l] < ts[m], -inf, S_T) / 32)
  attnT[vc,l] = v.T @ rel_T                  (256; partial over m-half)
768 matmuls/core total vs 1024 for the naive q/k formulation.

Host gathers: raw/rel halves are concatenated over m and transposed back;
attn halves are summed; out = concat([x, attn], -1). All matmuls in bf16
with fp32 PSUM accumulation; raw/rel/attn leave the chip as bf16. exp
needs no max-subtraction: scores/32 is ~N(0,1), far from fp32 range.
"""
import sys

sys.path.insert(0, "/opt/trn_rl_repo")

import numpy as np
import ml_dtypes
import bass_rust
import concourse.bass as bass
import concourse.mybir as mybir
import concourse.tile as tile
from concourse.bass import ts as bts
from concourse.bass_utils import run_bass_kernel_spmd

BF16 = mybir.dt.bfloat16
F32 = mybir.dt.float32
AF = mybir.ActivationFunctionType
OP = mybir.AluOpType
NPBF16 = ml_dtypes.bfloat16

B, L, D, K, V = 4, 2048, 1024, 1024, 1024
P = 128
MH = 1024                                   # m columns per core
DC, KC, MC, VC = D // P, K // P, MH // P, V // P   # all 8
LB = L // 512                               # 4 rhs blocks over l
SCALE = 1.0 / 32.0                          # 1/sqrt(K)

# ---------------------------------------------------------------- walrus fix
# This container's walrus build rejects instructions carrying more than one
# sync-wait command. Hoist excess waits onto injected same-engine nops
# (program order on the engine preserves wait-then-execute semantics).
_WAITFIX_UID = [0]


def _fix_sync_waits(nc, limit=1):
    for fn in nc.m.functions:
        for blk in fn.blocks:
            out = []
            for inst in blk.instructions:
                si = inst.sync_info
                waits = list(si.on_wait) if si is not None else []
                if len(waits) > limit:
                    extra, keep = waits[:-limit], waits[-limit:]
                    for j in range(0, len(extra), limit):
                        _WAITFIX_UID[0] += 1
                        nop = bass_rust.InstNoOp(
                            name=f"waitfix-{_WAITFIX_UID[0]}",
                            engine=inst.engine, ins=[], outs=[],
                        )
                        nop.sync_info = bass_rust.SyncInfo(
                            on_wait=extra[j:j + limit], on_update=[])
                        out.append(nop)
                    inst.sync_info = bass_rust.SyncInfo(
                        on_wait=keep, on_update=list(si.on_update))
                out.append(inst)
            blk.instructions = out


# ---------------------------------------------------------------- bass build
def _build_nc(loop_T=None):
    """loop_T=None: production program (phased pool release).
    loop_T=T: timing variant — body wrapped in a hardware For_i loop,
    flat pools, e/ev single-buffered to fit SBUF."""
    nc = bass.Bass()
    dp = nc.declare_dram_parameter
    xT = dp("xT", [D, L], BF16, isOutput=False)     # x[b].T, full l
    xTh = dp("xTh", [D, MH], BF16, isOutput=False)  # x[b].T, this half's m
    m_in = dp("m_in", [D, D], BF16, isOutput=False)   # M = Wk @ Wq.T
    wv = dp("wv", [D, V], BF16, isOutput=False)
    bv = dp("bv_bcast", [P, V], F32, isOutput=False)
    col = dp("col_cols", [P, MC], F32, isOutput=False)      # col corr
    colsc = dp("colsc_cols", [P, MC], F32, isOutput=False)  # col * SCALE
    row = dp("row_bcast", [P, L], F32, isOutput=False)      # row corr
    tsb = dp("ts_bcast", [P, L], F32, isOutput=False)
    tsc = dp("ts_cols", [P, MC], F32, isOutput=False)
    rawT = dp("rawT", [MH, L], BF16, isOutput=True)
    relT = dp("relT", [MH, L], BF16, isOutput=True)
    attnT = dp("attnT", [V, L], BF16, isOutput=True)

    timing = loop_T is not None
    wbufs = 1 if timing else 2

    with tile.TileContext(nc) as tc:
        import contextlib
        stack = contextlib.ExitStack()
        with stack:
            tp = lambda name, bufs, **kw: stack.enter_context(
                tc.tile_pool(name=name, bufs=bufs, **kw))
            p_const = tp("const", 1)
            p_x = tp("xp", 1)
            p_qkv = tp("qkv", 1)
            p_rel = tp("relp", 1)
            p_ps = tp("psum", 8, space="PSUM")

            tsb_sb = p_const.tile([P, L], F32)
            tsc_sb = p_const.tile([P, MC], F32)
            col_sb = p_const.tile([P, MC], F32)
            colsc_sb = p_const.tile([P, MC], F32)
            row_sb = p_const.tile([P, L], F32)
            bv_sb = p_const.tile([P, V], F32)
            nc.sync.dma_start(tsb_sb[:], tsb[:])
            nc.sync.dma_start(tsc_sb[:], tsc[:])
            nc.sync.dma_start(col_sb[:], col[:])
            nc.sync.dma_start(colsc_sb[:], colsc[:])
            nc.sync.dma_start(row_sb[:], row[:])
            nc.sync.dma_start(bv_sb[:], bv[:])

            xT_sb = p_x.tile([P, DC, L], BF16)
            xT_ap = xT.ap().rearrange("(o p) l -> p o l", p=P)

            gT_sb = p_qkv.tile([P, KC, MH], BF16)    # d'-chunks x m
            v_sb = p_qkv.tile([P, MC, V], BF16)      # m-chunks x vc
            rel_sb = p_rel.tile([P, MC, L], BF16)    # m-chunks x l

            if timing:
                p_in = tp("inp", 1)
                p_work = tp("work", wbufs)
                p_stat = tp("stat", 4)
            else:
                p_in = tc.alloc_tile_pool(name="inp", bufs=1)

            xTh_sb = p_in.tile([P, DC, MH], BF16)
            m_sb = p_in.tile([P, DC, D], BF16)
            wv_sb = p_in.tile([P, DC, V], BF16)
            xTh_ap = xTh.ap().rearrange("(o p) m -> p o m", p=P)
            m_ap = m_in.ap().rearrange("(o p) k -> p o k", p=P)
            wv_ap = wv.ap().rearrange("(o p) k -> p o k", p=P)
            for dc in range(DC):  # chunked so first matmuls start early
                nc.sync.dma_start(xT_sb[:, dc, :], xT_ap[:, dc, :])
                nc.sync.dma_start(xTh_sb[:, dc, :], xTh_ap[:, dc, :])
                nc.sync.dma_start(m_sb[:, dc, :], m_ap[:, dc, :])
                nc.sync.dma_start(wv_sb[:, dc, :], wv_ap[:, dc, :])

            loop_cm = tc.For_i(0, loop_T, 1) if timing else contextlib.nullcontext()
            with loop_cm:
                # gT[d', m] = M.T @ xTh
                for t in range(KC):
                    pss = [p_ps.tile([P, 512], F32, tag="ps", name="ps")
                           for _ in range(2)]
                    for dc in range(DC):
                        for mb in range(2):
                            nc.tensor.matmul(
                                pss[mb][:], m_sb[:, dc, bts(t, P)],
                                xTh_sb[:, dc, bts(mb, 512)],
                                start=(dc == 0), stop=(dc == DC - 1))
                    for mb in range(2):
                        nc.vector.tensor_copy(
                            gT_sb[:, t, bts(mb, 512)], pss[mb][:])

                # v[m, vc] = xTh.T @ Wv + bv
                for mc in range(MC):
                    pss = [p_ps.tile([P, 512], F32, tag="ps", name="ps")
                           for _ in range(2)]
                    for dc in range(DC):
                        for vb in range(2):
                            nc.tensor.matmul(
                                pss[vb][:], xTh_sb[:, dc, bts(mc, P)],
                                wv_sb[:, dc, bts(vb, 512)],
                                start=(dc == 0), stop=(dc == DC - 1))
                    for vb in range(2):
                        nc.vector.tensor_tensor(
                            v_sb[:, mc, bts(vb, 512)], pss[vb][:],
                            bv_sb[:, bts(vb, 512)], OP.add)

                if not timing:
                    # release input pool first (xTh/M/Wv consumed) so the
                    # work pools can reuse its address range
                    p_in.release()
                    p_work = stack.enter_context(
                        tc.tile_pool(name="work", bufs=wbufs))
                    p_stat = stack.enter_context(
                        tc.tile_pool(name="stat", bufs=4))

                # scores + masked softmax over l, per 128-row m-chunk
                for mc in range(MC):
                    pss = [p_ps.tile([P, 512], F32, tag="ps", name="ps")
                           for _ in range(LB)]
                    for t in range(KC):
                        for lb in range(LB):
                            nc.tensor.matmul(
                                pss[lb][:], gT_sb[:, t, bts(mc, P)],
                                xT_sb[:, t, bts(lb, 512)],
                                start=(t == 0), stop=(t == KC - 1))
                    raw_bf = p_work.tile([P, L], BF16, tag="rawbf", name="rawbf")
                    e_sb = p_work.tile([P, L], F32, tag="e", name="e")
                    for lb in range(LB):
                        # += row correction (zeros for zero biases)
                        nc.vector.tensor_tensor(
                            pss[lb][:], pss[lb][:], row_sb[:, bts(lb, 512)],
                            OP.add)
                        # raw = psum + col  (cast to bf16)
                        nc.vector.tensor_scalar(
                            raw_bf[:, bts(lb, 512)], pss[lb][:],
                            col_sb[:, mc:mc + 1], None, OP.add)
                        # e = exp(psum*1/32 + col*1/32)
                        nc.scalar.activation(
                            e_sb[:, bts(lb, 512)], pss[lb][:], AF.Exp,
                            bias=colsc_sb[:, mc:mc + 1], scale=SCALE)
                    nc.sync.dma_start(rawT[bts(mc, P), :], raw_bf[:])
                    ev_sb = p_work.tile([P, L], F32, tag="ev", name="ev")
                    rowsum = p_stat.tile([P, 1], F32, tag="rs", name="rs")
                    nc.vector.scalar_tensor_tensor(
                        ev_sb[:], tsb_sb[:], tsc_sb[:, mc:mc + 1], e_sb[:],
                        OP.is_ge, OP.mult, accum_out=rowsum[:])
                    rinv = p_stat.tile([P, 1], F32, tag="ri", name="ri")
                    nc.vector.reciprocal(rinv[:], rowsum[:])
                    nc.vector.tensor_scalar(
                        rel_sb[:, mc, :], ev_sb[:], rinv[:], None, OP.mult)
                    nc.sync.dma_start(relT[bts(mc, P), :], rel_sb[:, mc, :])

                # attention: attnT[vc, l] = sum_mc v[mc, vc].T @ rel_T[mc, l]
                for vc in range(VC):
                    pss = [p_ps.tile([P, 512], F32, tag="ps", name="ps")
                           for _ in range(LB)]
                    for mc in range(MC):
                        for lb in range(LB):
                            nc.tensor.matmul(
                                pss[lb][:], v_sb[:, mc, bts(vc, P)],
                                rel_sb[:, mc, bts(lb, 512)],
                                start=(mc == 0), stop=(mc == MC - 1))
                    at_bf = p_work.tile([P, L], BF16, tag="atbf", name="atbf")
                    for lb in range(LB):
                        nc.scalar.copy(at_bf[:, bts(lb, 512)], pss[lb][:])
                    nc.sync.dma_start(attnT[bts(vc, P), :], at_bf[:])

    _fix_sync_waits(nc)
    return nc


_CACHE = {}


def _get_nc():
    if "nc" not in _CACHE:
        _CACHE["nc"] = _build_nc()
    return _CACHE["nc"]


def make_in_maps(x, time_steps, Wq, bq, Wk, bk, Wv, bv):
    x = np.asarray(x, dtype=np.float32)
    tsf = np.asarray(time_steps).astype(np.float32)
    Wq = np.asarray(Wq, np.float32)
    Wk = np.asarray(Wk, np.float32)
    bq = np.asarray(bq, np.float32)
    bk = np.asarray(bk, np.float32)
    xT = np.ascontiguousarray(x.transpose(0, 2, 1)).astype(NPBF16)  # [B, D, L]
    m_np = (Wk @ Wq.T).astype(NPBF16)               # [D, D]
    wv_b = np.asarray(Wv, np.float32).astype(NPBF16)
    bv_bcast = np.ascontiguousarray(
        np.broadcast_to(np.asarray(bv, np.float32), (P, V)))
    # rank-1 bias corrections (zero when bq = bk = 0)
    colv = Wk @ bq                                   # [D]
    rowv = Wq @ bk                                   # [D]
    cc = float(bk @ bq)
    in_maps = []
    for c in range(8):
        b, h = c // 2, c % 2
        xb = x[b]
        col_c = xb[h * MH:(h + 1) * MH] @ colv + cc  # [MH]
        row_c = xb @ rowv                            # [L]
        col_cols = np.ascontiguousarray(col_c.reshape(MC, P).T.astype(np.float32))
        in_maps.append({
            "xT": xT[b],
            "xTh": np.ascontiguousarray(xT[b][:, h * MH:(h + 1) * MH]),
            "m_in": m_np, "wv": wv_b, "bv_bcast": bv_bcast,
            "col_cols": col_cols,
            "colsc_cols": np.ascontiguousarray(col_cols * SCALE),
            "row_bcast": np.ascontiguousarray(
                np.broadcast_to(row_c.astype(np.float32), (P, L))),
            "ts_bcast": np.ascontiguousarray(
                np.broadcast_to(tsf[b], (P, L))),
            "ts_cols": np.ascontiguousarray(
                tsf[b, h * MH:(h + 1) * MH].reshape(MC, P).T),
        })
    return in_maps


def assemble(x, results):
    x = np.asarray(x, dtype=np.float32)
    raw = np.empty((B, L, L), np.float32)
    rel = np.empty((B, L, L), np.float32)
    attn = np.zeros((B, L, V), np.float32)
    for c in range(8):
        b, h = c // 2, c % 2
        r = results[c]
        raw[b][:, h * MH:(h + 1) * MH] = r["rawT"].astype(np.float32).T
        rel[b][:, h * MH:(h + 1) * MH] = r["relT"].astype(np.float32).T
        attn[b] += r["attnT"].astype(np.float32).T
    out = np.concatenate([x, attn], axis=2)
    return out, rel, raw


def kernel(x, time_steps, Wq, bq, Wk, bk, Wv, bv):
    in_maps = make_in_maps(x, time_steps, Wq, bq, Wk, bk, Wv, bv)
    res = run_bass_kernel_spmd(_get_nc(), in_maps, list(range(8)))
    return assemble(x, res.results)


# revision 10
# speedup vs baseline: 1.0591x; 1.0359x over previous
"""Trainium2 Bass kernel for the masked single-head AttentionBlock.

Contract: kernel(**inputs) takes the FULL unsharded inputs from
reference.setup_inputs() and returns (out, rel, raw) exactly like
reference.reference().

Sharding: 8 NeuronCores = 4 batches x 2 key-column halves. Core c handles
(b = c//2, half = c%2).

Math: q and k only ever appear through raw = q @ k.T, so with
M = Wk @ Wq.T (computed once on the host in fp32):
  raw.T[m, l] = x[m] @ M @ x[l].T  + col[m] + row[l]
where col[m] = x[m]@(Wk bq) + bk@bq and row[l] = x[l]@(Wq bk) are host-
computed rank-1 bias corrections (identically zero for this problem's
zero biases, but kept for generality: row is added to the score PSUM by
the DVE, col rides the existing per-partition bias slots of the raw-cast
and exp instructions).

Per core:
  gT[d', m]   = M.T @ xTh                    (128 matmuls)
  v[m, vc]    = xTh.T @ Wv + bv              (128)
  S_T[m, l]   = gT.T @ xT  (+row +col)       (256; softmax axis l is free)
  rel_T[m, l] = softmax_l(where(ts[l] < ts[m], -inf, S_T) / 32)
  attnT[vc,l] = v.T @ rel_T                  (256; partial over m-half)
768 matmuls/core total vs 1024 for the naive q/k formulation.

Host gathers: raw/rel halves are concatenated over m and transposed back;
attn halves are summed; out = concat([x, attn], -1). All matmuls in bf16
with fp32 PSUM accumulation; raw/rel/attn leave the chip as bf16. exp
needs no max-subtraction: scores/32 is ~N(0,1), far from fp32 range.
"""
import sys

sys.path.insert(0, "/opt/trn_rl_repo")

import numpy as np
import ml_dtypes
import bass_rust
import concourse.bass as bass
import concourse.mybir as mybir
import concourse.tile as tile
from concourse.bass import ts as bts
from concourse.bass_utils import run_bass_kernel_spmd

BF16 = mybir.dt.bfloat16
F32 = mybir.dt.float32
AF = mybir.ActivationFunctionType
OP = mybir.AluOpType
NPBF16 = ml_dtypes.bfloat16

B, L, D, K, V = 4, 2048, 1024, 1024, 1024
P = 128
MH = 1024                                   # m columns per core
DC, KC, MC, VC = D // P, K // P, MH // P, V // P   # all 8
LB = L // 512                               # 4 rhs blocks over l
SCALE = 1.0 / 32.0                          # 1/sqrt(K)

# ---------------------------------------------------------------- walrus fix
# This container's walrus build rejects instructions carrying more than one
# sync-wait command. Hoist excess waits onto injected same-engine nops
# (program order on the engine preserves wait-then-execute semantics).
_WAITFIX_UID = [0]


def _fix_sync_waits(nc, limit=1):
    for fn in nc.m.functions:
        for blk in fn.blocks:
            out = []
            for inst in blk.instructions:
                si = inst.sync_info
                waits = list(si.on_wait) if si is not None else []
                if len(waits) > limit:
                    extra, keep = waits[:-limit], waits[-limit:]
                    for j in range(0, len(extra), limit):
                        _WAITFIX_UID[0] += 1
                        nop = bass_rust.InstNoOp(
                            name=f"waitfix-{_WAITFIX_UID[0]}",
                            engine=inst.engine, ins=[], outs=[],
                        )
                        nop.sync_info = bass_rust.SyncInfo(
                            on_wait=extra[j:j + limit], on_update=[])
                        out.append(nop)
                    inst.sync_info = bass_rust.SyncInfo(
                        on_wait=keep, on_update=list(si.on_update))
                out.append(inst)
            blk.instructions = out


# ---------------------------------------------------------------- bass build
def _build_nc(loop_T=None):
    """loop_T=None: production program (phased pool release).
    loop_T=T: timing variant — body wrapped in a hardware For_i loop,
    flat pools, e/ev single-buffered to fit SBUF."""
    nc = bass.Bass()
    dp = nc.declare_dram_parameter
    xT = dp("xT", [D, L], BF16, isOutput=False)     # x[b].T, full l
    xTh = dp("xTh", [D, MH], BF16, isOutput=False)  # x[b].T, this half's m
    m_in = dp("m_in", [D, D], BF16, isOutput=False)   # M = Wk @ Wq.T
    wv = dp("wv", [D, V], BF16, isOutput=False)
    bv = dp("bv_bcast", [P, V], F32, isOutput=False)
    col = dp("col_cols", [P, MC], F32, isOutput=False)      # col corr
    colsc = dp("colsc_cols", [P, MC], F32, isOutput=False)  # col * SCALE
    row = dp("row_bcast", [P, L], F32, isOutput=False)      # row corr
    tsb = dp("ts_bcast", [P, L], F32, isOutput=False)
    tsc = dp("ts_cols", [P, MC], F32, isOutput=False)
    rawT = dp("rawT", [MH, L], BF16, isOutput=True)
    relT = dp("relT", [MH, L], BF16, isOutput=True)
    attnT = dp("attnT", [V, L], BF16, isOutput=True)

    timing = loop_T is not None
    if timing:
        cnt = dp("cnt", [P, 1], F32, isOutput=True)
    wbufs = 1 if timing else 2

    with tile.TileContext(nc) as tc:
        import contextlib
        stack = contextlib.ExitStack()
        with stack:
            tp = lambda name, bufs, **kw: stack.enter_context(
                tc.tile_pool(name=name, bufs=bufs, **kw))
            p_const = tp("const", 1)
            p_x = tp("xp", 1)
            p_qkv = tp("qkv", 1)
            p_rel = tp("relp", 1)
            p_ps = tp("psum", 8, space="PSUM")

            tsb_sb = p_const.tile([P, L], F32)
            tsc_sb = p_const.tile([P, MC], F32)
            col_sb = p_const.tile([P, MC], F32)
            colsc_sb = p_const.tile([P, MC], F32)
            row_sb = p_const.tile([P, L], F32)
            bv_sb = p_const.tile([P, V], F32)
            nc.sync.dma_start(tsb_sb[:], tsb[:])
            nc.sync.dma_start(tsc_sb[:], tsc[:])
            nc.sync.dma_start(col_sb[:], col[:])
            nc.sync.dma_start(colsc_sb[:], colsc[:])
            nc.sync.dma_start(row_sb[:], row[:])
            nc.sync.dma_start(bv_sb[:], bv[:])

            xT_sb = p_x.tile([P, DC, L], BF16)
            xT_ap = xT.ap().rearrange("(o p) l -> p o l", p=P)

            gT_sb = p_qkv.tile([P, KC, MH], BF16)    # d'-chunks x m
            v_sb = p_qkv.tile([P, MC, V], BF16)      # m-chunks x vc
            rel_sb = p_rel.tile([P, MC, L], BF16)    # m-chunks x l

            if timing:
                p_in = tp("inp", 1)
                p_work = tp("work", wbufs)
                p_stat = tp("stat", 4)
            else:
                p_in = tc.alloc_tile_pool(name="inp", bufs=1)

            xTh_sb = p_in.tile([P, DC, MH], BF16)
            m_sb = p_in.tile([P, DC, D], BF16)
            wv_sb = p_in.tile([P, DC, V], BF16)
            xTh_ap = xTh.ap().rearrange("(o p) m -> p o m", p=P)
            m_ap = m_in.ap().rearrange("(o p) k -> p o k", p=P)
            wv_ap = wv.ap().rearrange("(o p) k -> p o k", p=P)
            # chunked loads; xTh/M feed the first matmuls (gT), wv the second
            # phase (V), and xT is not read until the scores phase, so it
            # streams last.
            for dc in range(DC):
                nc.sync.dma_start(xTh_sb[:, dc, :], xTh_ap[:, dc, :])
                nc.sync.dma_start(m_sb[:, dc, :], m_ap[:, dc, :])
            for dc in range(DC):
                nc.sync.dma_start(wv_sb[:, dc, :], wv_ap[:, dc, :])
            for dc in range(DC):
                nc.sync.dma_start(xT_sb[:, dc, :], xT_ap[:, dc, :])

            if timing:
                cnt_sb = p_const.tile([P, 1], F32)
                nc.gpsimd.memset(cnt_sb[:], 0.0)
            loop_cm = tc.For_i(0, loop_T, 1) if timing else contextlib.nullcontext()
            with loop_cm:
                if timing:
                    nc.vector.tensor_scalar(
                        cnt_sb[:], cnt_sb[:], 1.0, None, OP.add)
                # gT[d', m] = M.T @ xTh
                for t in range(KC):
                    pss = [p_ps.tile([P, 512], F32, tag="ps", name="ps")
                           for _ in range(2)]
                    for dc in range(DC):
                        for mb in range(2):
                            nc.tensor.matmul(
                                pss[mb][:], m_sb[:, dc, bts(t, P)],
                                xTh_sb[:, dc, bts(mb, 512)],
                                start=(dc == 0), stop=(dc == DC - 1))
                    for mb in range(2):
                        nc.vector.tensor_copy(
                            gT_sb[:, t, bts(mb, 512)], pss[mb][:])

                # v[m, vc] = xTh.T @ Wv + bv
                for mc in range(MC):
                    pss = [p_ps.tile([P, 512], F32, tag="ps", name="ps")
                           for _ in range(2)]
                    for dc in range(DC):
                        for vb in range(2):
                            nc.tensor.matmul(
                                pss[vb][:], xTh_sb[:, dc, bts(mc, P)],
                                wv_sb[:, dc, bts(vb, 512)],
                                start=(dc == 0), stop=(dc == DC - 1))
                    for vb in range(2):
                        nc.vector.tensor_tensor(
                            v_sb[:, mc, bts(vb, 512)], pss[vb][:],
                            bv_sb[:, bts(vb, 512)], OP.add)

                if not timing:
                    # release input pool first (xTh/M/Wv consumed) so the
                    # work pools can reuse its address range
                    p_in.release()
                    p_work = stack.enter_context(
                        tc.tile_pool(name="work", bufs=wbufs))
                    p_stat = stack.enter_context(
                        tc.tile_pool(name="stat", bufs=4))

                # scores + masked softmax over l, per 128-row m-chunk
                for mc in range(MC):
                    pss = [p_ps.tile([P, 512], F32, tag="ps", name="ps")
                           for _ in range(LB)]
                    for t in range(KC):
                        for lb in range(LB):
                            nc.tensor.matmul(
                                pss[lb][:], gT_sb[:, t, bts(mc, P)],
                                xT_sb[:, t, bts(lb, 512)],
                                start=(t == 0), stop=(t == KC - 1))
                    raw_bf = p_work.tile([P, L], BF16, tag="rawbf", name="rawbf")
                    e_sb = p_work.tile([P, L], F32, tag="e", name="e")
                    for lb in range(LB):
                        # += row correction (zeros for zero biases)
                        nc.vector.tensor_tensor(
                            pss[lb][:], pss[lb][:], row_sb[:, bts(lb, 512)],
                            OP.add)
                        # raw = psum + col  (cast to bf16)
                        nc.vector.tensor_scalar(
                            raw_bf[:, bts(lb, 512)], pss[lb][:],
                            col_sb[:, mc:mc + 1], None, OP.add)
                        # e = exp(psum*1/32 + col*1/32)
                        nc.scalar.activation(
                            e_sb[:, bts(lb, 512)], pss[lb][:], AF.Exp,
                            bias=colsc_sb[:, mc:mc + 1], scale=SCALE)
                    nc.sync.dma_start(rawT[bts(mc, P), :], raw_bf[:])
                    ev_sb = p_work.tile([P, L], F32, tag="ev", name="ev")
                    rowsum = p_stat.tile([P, 1], F32, tag="rs", name="rs")
                    nc.vector.scalar_tensor_tensor(
                        ev_sb[:], tsb_sb[:], tsc_sb[:, mc:mc + 1], e_sb[:],
                        OP.is_ge, OP.mult, accum_out=rowsum[:])
                    rinv = p_stat.tile([P, 1], F32, tag="ri", name="ri")
                    nc.vector.reciprocal(rinv[:], rowsum[:])
                    nc.vector.tensor_scalar(
                        rel_sb[:, mc, :], ev_sb[:], rinv[:], None, OP.mult)
                    nc.sync.dma_start(relT[bts(mc, P), :], rel_sb[:, mc, :])

                # attention: attnT[vc, l] = sum_mc v[mc, vc].T @ rel_T[mc, l]
                for vc in range(VC):
                    pss = [p_ps.tile([P, 512], F32, tag="ps", name="ps")
                           for _ in range(LB)]
                    for mc in range(MC):
                        for lb in range(LB):
                            nc.tensor.matmul(
                                pss[lb][:], v_sb[:, mc, bts(vc, P)],
                                rel_sb[:, mc, bts(lb, 512)],
                                start=(mc == 0), stop=(mc == MC - 1))
                    at_bf = p_work.tile([P, L], BF16, tag="atbf", name="atbf")
                    for lb in range(LB):
                        nc.scalar.copy(at_bf[:, bts(lb, 512)], pss[lb][:])
                    nc.sync.dma_start(attnT[bts(vc, P), :], at_bf[:])

            if timing:
                nc.sync.dma_start(cnt[:], cnt_sb[:])

    _fix_sync_waits(nc)
    return nc


_CACHE = {}


def _get_nc():
    if "nc" not in _CACHE:
        _CACHE["nc"] = _build_nc()
    return _CACHE["nc"]


def make_in_maps(x, time_steps, Wq, bq, Wk, bk, Wv, bv):
    x = np.asarray(x, dtype=np.float32)
    tsf = np.asarray(time_steps).astype(np.float32)
    Wq = np.asarray(Wq, np.float32)
    Wk = np.asarray(Wk, np.float32)
    bq = np.asarray(bq, np.float32)
    bk = np.asarray(bk, np.float32)
    xT = np.ascontiguousarray(x.transpose(0, 2, 1)).astype(NPBF16)  # [B, D, L]
    m_np = (Wk @ Wq.T).astype(NPBF16)               # [D, D]
    wv_b = np.asarray(Wv, np.float32).astype(NPBF16)
    bv_bcast = np.ascontiguousarray(
        np.broadcast_to(np.asarray(bv, np.float32), (P, V)))
    # rank-1 bias corrections (zero when bq = bk = 0)
    colv = Wk @ bq                                   # [D]
    rowv = Wq @ bk                                   # [D]
    cc = float(bk @ bq)
    in_maps = []
    for c in range(8):
        b, h = c // 2, c % 2
        xb = x[b]
        col_c = xb[h * MH:(h + 1) * MH] @ colv + cc  # [MH]
        row_c = xb @ rowv                            # [L]
        col_cols = np.ascontiguousarray(col_c.reshape(MC, P).T.astype(np.float32))
        in_maps.append({
            "xT": xT[b],
            "xTh": np.ascontiguousarray(xT[b][:, h * MH:(h + 1) * MH]),
            "m_in": m_np, "wv": wv_b, "bv_bcast": bv_bcast,
            "col_cols": col_cols,
            "colsc_cols": np.ascontiguousarray(col_cols * SCALE),
            "row_bcast": np.ascontiguousarray(
                np.broadcast_to(row_c.astype(np.float32), (P, L))),
            "ts_bcast": np.ascontiguousarray(
                np.broadcast_to(tsf[b], (P, L))),
            "ts_cols": np.ascontiguousarray(
                tsf[b, h * MH:(h + 1) * MH].reshape(MC, P).T),
        })
    return in_maps


def assemble(x, results):
    x = np.asarray(x, dtype=np.float32)
    raw = np.empty((B, L, L), np.float32)
    rel = np.empty((B, L, L), np.float32)
    attn = np.zeros((B, L, V), np.float32)
    for c in range(8):
        b, h = c // 2, c % 2
        r = results[c]
        raw[b][:, h * MH:(h + 1) * MH] = r["rawT"].astype(np.float32).T
        rel[b][:, h * MH:(h + 1) * MH] = r["relT"].astype(np.float32).T
        attn[b] += r["attnT"].astype(np.float32).T
    out = np.concatenate([x, attn], axis=2)
    return out, rel, raw


def kernel(x, time_steps, Wq, bq, Wk, bk, Wv, bv):
    in_maps = make_in_maps(x, time_steps, Wq, bq, Wk, bk, Wv, bv)
    res = run_bass_kernel_spmd(_get_nc(), in_maps, list(range(8)))
    return assemble(x, res.results)


# revision 16
# speedup vs baseline: 34413.8167x; 32493.0447x over previous
"""Trainium2 Bass kernel for the masked single-head AttentionBlock.

Contract: kernel(**inputs) takes the FULL unsharded inputs from
reference.setup_inputs() and returns (out, rel, raw) exactly like
reference.reference().

Sharding: 8 NeuronCores = 4 batches x 2 key-column halves. Core c handles
(b = c//2, half = c%2).

Math: q and k only ever appear through raw = q @ k.T, so with
M = Wk @ Wq.T (computed once on the host in fp32):
  raw.T[m, l] = x[m] @ M @ x[l].T  + col[m] + row[l]
where col[m] = x[m]@(Wk bq) + bk@bq and row[l] = x[l]@(Wq bk) are host-
computed rank-1 bias corrections (identically zero for this problem's
zero biases, but kept for generality: row is added to the score PSUM by
the DVE, col rides the existing per-partition bias slots of the raw-cast
and exp instructions).

Per core:
  gT[d', m]   = M.T @ xTh                    (128 matmuls)
  v[m, vc]    = xTh.T @ Wv + bv              (128)
  S_T[m, l]   = gT.T @ xT  (+row +col)       (256; softmax axis l is free)
  rel_T[m, l] = softmax_l(where(ts[l] < ts[m], -inf, S_T) / 32)
  attnT[vc,l] = v.T @ rel_T                  (256; partial over m-half)
768 matmuls/core total vs 1024 for the naive q/k formulation.

Host gathers: raw/rel halves are concatenated over m and transposed back;
attn halves are summed; out = concat([x, attn], -1). All matmuls in bf16
with fp32 PSUM accumulation; raw/rel/attn leave the chip as bf16. exp
needs no max-subtraction: scores/32 is ~N(0,1), far from fp32 range.
"""
import sys

sys.path.insert(0, "/opt/trn_rl_repo")

import numpy as np
import ml_dtypes
import bass_rust
import concourse.bass as bass
import concourse.mybir as mybir
import concourse.tile as tile
from concourse.bass import ts as bts
from concourse.bass_utils import run_bass_kernel_spmd

BF16 = mybir.dt.bfloat16
F32 = mybir.dt.float32
AF = mybir.ActivationFunctionType
OP = mybir.AluOpType
NPBF16 = ml_dtypes.bfloat16

B, L, D, K, V = 4, 2048, 1024, 1024, 1024
P = 128
MH = 1024                                   # m columns per core
DC, KC, MC, VC = D // P, K // P, MH // P, V // P   # all 8
LB = L // 512                               # 4 rhs blocks over l
SCALE = 1.0 / 32.0                          # 1/sqrt(K)
ATTN_FP8 = False                           # fp8e4m3+DoubleRow attn phase

# ---------------------------------------------------------------- walrus fix
# This container's walrus build rejects instructions carrying more than one
# sync-wait command. Hoist excess waits onto injected same-engine nops
# (program order on the engine preserves wait-then-execute semantics).
_WAITFIX_UID = [0]


def _fix_sync_waits(nc, limit=1):
    for fn in nc.m.functions:
        for blk in fn.blocks:
            out = []
            for inst in blk.instructions:
                si = inst.sync_info
                waits = list(si.on_wait) if si is not None else []
                if len(waits) > limit:
                    extra, keep = waits[:-limit], waits[-limit:]
                    for j in range(0, len(extra), limit):
                        _WAITFIX_UID[0] += 1
                        nop = bass_rust.InstNoOp(
                            name=f"waitfix-{_WAITFIX_UID[0]}",
                            engine=inst.engine, ins=[], outs=[],
                        )
                        nop.sync_info = bass_rust.SyncInfo(
                            on_wait=extra[j:j + limit], on_update=[])
                        out.append(nop)
                    inst.sync_info = bass_rust.SyncInfo(
                        on_wait=keep, on_update=list(si.on_update))
                out.append(inst)
            blk.instructions = out


# ---------------------------------------------------------------- bass build
def _build_nc(loop_T=None):
    """loop_T=None: production program (phased pool release).
    loop_T=T: timing variant — body wrapped in a hardware For_i loop,
    flat pools, e/ev single-buffered to fit SBUF."""
    nc = bass.Bass()
    dp = nc.declare_dram_parameter
    xT = dp("xT", [D, L], BF16, isOutput=False)     # x[b].T, full l
    xTh = dp("xTh", [D, MH], BF16, isOutput=False)  # x[b].T, this half's m
    m_in = dp("m_in", [D, D], BF16, isOutput=False)   # M = Wk @ Wq.T
    wv = dp("wv", [D, V], BF16, isOutput=False)
    bv = dp("bv_bcast", [P, V], F32, isOutput=False)
    col = dp("col_cols", [P, MC], F32, isOutput=False)      # col corr
    colsc = dp("colsc_cols", [P, MC], F32, isOutput=False)  # col * SCALE
    row = dp("row_bcast", [P, L], F32, isOutput=False)      # row corr
    tsb = dp("ts_bcast", [P, L], F32, isOutput=False)
    tsc = dp("ts_cols", [P, MC], F32, isOutput=False)
    rawT = dp("rawT", [MH, L], BF16, isOutput=True)
    relT = dp("relT", [MH, L], BF16, isOutput=True)
    attnT = dp("attnT", [V, L], BF16, isOutput=True)

    timing = loop_T is not None
    if timing:
        cnt = dp("cnt", [P, 1], F32, isOutput=True)
    wbufs = 1 if timing else 2

    with tile.TileContext(nc) as tc:
        import contextlib
        stack = contextlib.ExitStack()
        with stack:
            tp = lambda name, bufs, **kw: stack.enter_context(
                tc.tile_pool(name=name, bufs=bufs, **kw))
            p_const = tp("const", 1)
            p_x = tp("xp", 1)
            p_qkv = tp("qkv", 1)
            p_rel = tp("relp", 1)
            p_ps = tp("psum", 8, space="PSUM")

            tsb_sb = p_const.tile([P, L], F32)
            tsc_sb = p_const.tile([P, MC], F32)
            col_sb = p_const.tile([P, MC], F32)
            colsc_sb = p_const.tile([P, MC], F32)
            row_sb = p_const.tile([P, L], F32)
            bv_sb = p_const.tile([P, V], F32)
            nc.sync.dma_start(tsb_sb[:], tsb[:])
            nc.sync.dma_start(tsc_sb[:], tsc[:])
            nc.sync.dma_start(col_sb[:], col[:])
            nc.sync.dma_start(colsc_sb[:], colsc[:])
            nc.sync.dma_start(row_sb[:], row[:])
            nc.sync.dma_start(bv_sb[:], bv[:])

            xT_sb = p_x.tile([P, DC, L], BF16)
            xT_ap = xT.ap().rearrange("(o p) l -> p o l", p=P)

            gT_sb = p_qkv.tile([P, KC, MH], BF16)    # d'-chunks x m
            v_sb = p_qkv.tile([P, MC, V], BF16)      # m-chunks x vc
            rel_sb = p_rel.tile([P, MC, L], BF16)    # m-chunks x l
            if ATTN_FP8:
                FP8 = mybir.dt.float8e4
                v8_sb = p_qkv.tile([P, MC, V], FP8)
                rel8_sb = p_rel.tile([P, MC, L], FP8)

            if timing:
                p_in = tp("inp", 1)
                p_work = tp("work", wbufs)
                p_stat = tp("stat", 4)
            else:
                p_in = tc.alloc_tile_pool(name="inp", bufs=1)

            xTh_sb = p_in.tile([P, DC, MH], BF16)
            m_sb = p_in.tile([P, DC, D], BF16)
            wv_sb = p_in.tile([P, DC, V], BF16)
            xTh_ap = xTh.ap().rearrange("(o p) m -> p o m", p=P)
            m_ap = m_in.ap().rearrange("(o p) k -> p o k", p=P)
            wv_ap = wv.ap().rearrange("(o p) k -> p o k", p=P)
            # chunked loads; xTh/M feed the first matmuls (gT), wv the second
            # phase (V), and xT is not read until the scores phase, so it
            # streams last.
            for dc in range(DC):
                nc.sync.dma_start(xTh_sb[:, dc, :], xTh_ap[:, dc, :])
                nc.sync.dma_start(m_sb[:, dc, :], m_ap[:, dc, :])
            for dc in range(DC):
                nc.sync.dma_start(wv_sb[:, dc, :], wv_ap[:, dc, :])
            for dc in range(DC):
                nc.sync.dma_start(xT_sb[:, dc, :], xT_ap[:, dc, :])

            if timing:
                cnt_sb = p_const.tile([P, 1], F32)
                nc.gpsimd.memset(cnt_sb[:], 0.0)
            loop_cm = tc.For_i(0, loop_T, 1) if timing else contextlib.nullcontext()
            with loop_cm:
                if timing:
                    nc.vector.tensor_scalar(
                        cnt_sb[:], cnt_sb[:], 1.0, None, OP.add)
                # gT[d', m] = M.T @ xTh
                for t in range(KC):
                    pss = [p_ps.tile([P, 512], F32, tag="ps", name="ps")
                           for _ in range(2)]
                    for dc in range(DC):
                        for mb in range(2):
                            nc.tensor.matmul(
                                pss[mb][:], m_sb[:, dc, bts(t, P)],
                                xTh_sb[:, dc, bts(mb, 512)],
                                start=(dc == 0), stop=(dc == DC - 1))
                    for mb in range(2):
                        nc.vector.tensor_copy(
                            gT_sb[:, t, bts(mb, 512)], pss[mb][:])

                # v[m, vc] = xTh.T @ Wv + bv
                for mc in range(MC):
                    pss = [p_ps.tile([P, 512], F32, tag="ps", name="ps")
                           for _ in range(2)]
                    for dc in range(DC):
                        for vb in range(2):
                            nc.tensor.matmul(
                                pss[vb][:], xTh_sb[:, dc, bts(mc, P)],
                                wv_sb[:, dc, bts(vb, 512)],
                                start=(dc == 0), stop=(dc == DC - 1))
                    for vb in range(2):
                        nc.vector.tensor_tensor(
                            v_sb[:, mc, bts(vb, 512)], pss[vb][:],
                            bv_sb[:, bts(vb, 512)], OP.add)
                    if ATTN_FP8:
                        nc.vector.tensor_copy(v8_sb[:, mc, :], v_sb[:, mc, :])

                if not timing:
                    # release input pool first (xTh/M/Wv consumed) so the
                    # work pools can reuse its address range
                    p_in.release()
                    p_work = stack.enter_context(
                        tc.tile_pool(name="work", bufs=wbufs))
                    p_stat = stack.enter_context(
                        tc.tile_pool(name="stat", bufs=4))

                # scores + masked softmax over l, per 128-row m-chunk
                for mc in range(MC):
                    pss = [p_ps.tile([P, 512], F32, tag="ps", name="ps")
                           for _ in range(LB)]
                    for t in range(KC):
                        for lb in range(LB):
                            nc.tensor.matmul(
                                pss[lb][:], gT_sb[:, t, bts(mc, P)],
                                xT_sb[:, t, bts(lb, 512)],
                                start=(t == 0), stop=(t == KC - 1))
                    raw_bf = p_work.tile([P, L], BF16, tag="rawbf", name="rawbf")
                    e_sb = p_work.tile([P, L], F32, tag="e", name="e")
                    for lb in range(LB):
                        # += row correction (zeros for zero biases)
                        nc.vector.tensor_tensor(
                            pss[lb][:], pss[lb][:], row_sb[:, bts(lb, 512)],
                            OP.add)
                        # raw = psum + col  (cast to bf16)
                        nc.vector.tensor_scalar(
                            raw_bf[:, bts(lb, 512)], pss[lb][:],
                            col_sb[:, mc:mc + 1], None, OP.add)
                        # e = exp(psum*1/32 + col*1/32)
                        nc.scalar.activation(
                            e_sb[:, bts(lb, 512)], pss[lb][:], AF.Exp,
                            bias=colsc_sb[:, mc:mc + 1], scale=SCALE)
                    nc.sync.dma_start(rawT[bts(mc, P), :], raw_bf[:])
                    ev_sb = p_work.tile([P, L], F32, tag="ev", name="ev")
                    rowsum = p_stat.tile([P, 1], F32, tag="rs", name="rs")
                    nc.vector.scalar_tensor_tensor(
                        ev_sb[:], tsb_sb[:], tsc_sb[:, mc:mc + 1], e_sb[:],
                        OP.is_ge, OP.mult, accum_out=rowsum[:])
                    rinv = p_stat.tile([P, 1], F32, tag="ri", name="ri")
                    nc.vector.reciprocal(rinv[:], rowsum[:])
                    nc.vector.tensor_scalar(
                        rel_sb[:, mc, :], ev_sb[:], rinv[:], None, OP.mult)
                    nc.sync.dma_start(relT[bts(mc, P), :], rel_sb[:, mc, :])
                    if ATTN_FP8:
                        nc.vector.tensor_copy(rel8_sb[:, mc, :], rel_sb[:, mc, :])

                # attention: attnT[vc, l] = sum_mc v[mc, vc].T @ rel_T[mc, l]
                for vc in range(VC):
                    pss = [p_ps.tile([P, 512], F32, tag="ps", name="ps")
                           for _ in range(LB)]
                    if ATTN_FP8:
                        # DoubleRow: two m-chunks per matmul via [Ki, 2, N] APs
                        for md in range(MC // 2):
                            for lb in range(LB):
                                nc.tensor.matmul(
                                    pss[lb][:],
                                    v8_sb[:, 2 * md:2 * md + 2, bts(vc, P)],
                                    rel8_sb[:, 2 * md:2 * md + 2, bts(lb, 512)],
                                    start=(md == 0), stop=(md == MC // 2 - 1),
                                    perf_mode=mybir.MatmulPerfMode.DoubleRow)
                    else:
                        for mc in range(MC):
                            for lb in range(LB):
                                nc.tensor.matmul(
                                    pss[lb][:], v_sb[:, mc, bts(vc, P)],
                                    rel_sb[:, mc, bts(lb, 512)],
                                    start=(mc == 0), stop=(mc == MC - 1))
                    at_bf = p_work.tile([P, L], BF16, tag="atbf", name="atbf")
                    for lb in range(LB):
                        nc.scalar.copy(at_bf[:, bts(lb, 512)], pss[lb][:])
                    nc.sync.dma_start(attnT[bts(vc, P), :], at_bf[:])

            if timing:
                nc.sync.dma_start(cnt[:], cnt_sb[:])

    _fix_sync_waits(nc)
    return nc


_CACHE = {}


def _get_nc():
    if "nc" not in _CACHE:
        _CACHE["nc"] = _build_nc()
    return _CACHE["nc"]


def make_in_maps(x, time_steps, Wq, bq, Wk, bk, Wv, bv):
    x = np.asarray(x, dtype=np.float32)
    tsf = np.asarray(time_steps).astype(np.float32)
    Wq = np.asarray(Wq, np.float32)
    Wk = np.asarray(Wk, np.float32)
    bq = np.asarray(bq, np.float32)
    bk = np.asarray(bk, np.float32)
    xT = np.ascontiguousarray(x.transpose(0, 2, 1)).astype(NPBF16)  # [B, D, L]
    m_np = (Wk @ Wq.T).astype(NPBF16)               # [D, D]
    wv_b = np.asarray(Wv, np.float32).astype(NPBF16)
    bv_bcast = np.ascontiguousarray(
        np.broadcast_to(np.asarray(bv, np.float32), (P, V)))
    # rank-1 bias corrections (zero when bq = bk = 0)
    colv = Wk @ bq                                   # [D]
    rowv = Wq @ bk                                   # [D]
    cc = float(bk @ bq)
    in_maps = []
    for c in range(8):
        b, h = c // 2, c % 2
        xb = x[b]
        col_c = xb[h * MH:(h + 1) * MH] @ colv + cc  # [MH]
        row_c = xb @ rowv                            # [L]
        col_cols = np.ascontiguousarray(col_c.reshape(MC, P).T.astype(np.float32))
        in_maps.append({
            "xT": xT[b],
            "xTh": np.ascontiguousarray(xT[b][:, h * MH:(h + 1) * MH]),
            "m_in": m_np, "wv": wv_b, "bv_bcast": bv_bcast,
            "col_cols": col_cols,
            "colsc_cols": np.ascontiguousarray(col_cols * SCALE),
            "row_bcast": np.ascontiguousarray(
                np.broadcast_to(row_c.astype(np.float32), (P, L))),
            "ts_bcast": np.ascontiguousarray(
                np.broadcast_to(tsf[b], (P, L))),
            "ts_cols": np.ascontiguousarray(
                tsf[b, h * MH:(h + 1) * MH].reshape(MC, P).T),
        })
    return in_maps


def assemble(x, results):
    x = np.asarray(x, dtype=np.float32)
    raw = np.empty((B, L, L), np.float32)
    rel = np.empty((B, L, L), np.float32)
    attn = np.zeros((B, L, V), np.float32)
    for c in range(8):
        b, h = c // 2, c % 2
        r = results[c]
        raw[b][:, h * MH:(h + 1) * MH] = r["rawT"].astype(np.float32).T
        rel[b][:, h * MH:(h + 1) * MH] = r["relT"].astype(np.float32).T
        attn[b] += r["attnT"].astype(np.float32).T
    out = np.concatenate([x, attn], axis=2)
    return out, rel, raw


def kernel(x, time_steps, Wq, bq, Wk, bk, Wv, bv):
    in_maps = make_in_maps(x, time_steps, Wq, bq, Wk, bk, Wv, bv)
    res = run_bass_kernel_spmd(_get_nc(), in_maps, list(range(8)))
    return assemble(x, res.results)
